# revision 7
# baseline (speedup 1.0000x reference)
"""Entmax-1.5 (bisection reference) kernel for Trainium2, 8-core data parallel.

The reference runs 50 bisection iterations on tau with bracket
[min(xs)-1, max(xs)=0], xs = x - rowmax(x), z = 0.5*xs,
y = clip(z - tau, 0)^2, constraint = sum(y) - 1, and the update
  tmin = where(constraint < 0, tau, tmin)
  tmax = where(constraint > 0, tau, tmax)
For any row of width N >= 5 the first midpoint tau_1 = (min(xs)-1)/2
satisfies z_i - tau_1 = (xs_i - min(xs) + 1)/2 >= 1/2 for every i, so
constraint >= N/4 - 1 > 0 at tau_1 and at every later (smaller) tau.
Only tmax ever updates, and the f32 halving sequence collapses onto
tmin = min(xs) - 1 within ~30 iterations. Hence the reference equals

    w_i = (0.5*x_i + b)^2,  b = 0.5*rowmax(x) - rowmin(x) + 1
    out = w / (rowsum(w) + 1e-12)

(verified numerically: 5e-7 elementwise relative vs the 50-iter loop).

Kernel per core (512 rows x 32000 cols f32), per 128-row chunk:
  stats phase (overlapped with the loads, per column tile):
    DVE  tensor_scalar x+0 in place, accum -> sum(x)      (2x mode)
    DVE  reduce_max / reduce_min
    ACT  Square(0.5x) into a bf16 trash tile, accum -> 0.25*sum(x^2)
  then S = 0.25*sum(x^2) + b*sum(x) + N*b^2 + 1e-12, q = sqrt(1/S),
  and a SINGLE fused ACT pass out = (0.5q*x + q*b)^2 = w/S in place,
  stored as it completes. One read + one write of HBM, DMA-bound.
"""

import numpy as np

N_CORES = 8
ROWS, COLS = 4096, 32000
RPC = ROWS // N_CORES  # rows per core
P = 128  # SBUF partitions
WTILE = 4000  # column tile width
XBUFS = 10  # x-tile slots (each 128 x WTILE f32)


def _build(rows, cols, wtile, xbufs=XBUFS):
    import concourse.bass as bass
    import concourse.tile as tile
    from concourse import bacc, mybir

    f32 = mybir.dt.float32
    bf16 = mybir.dt.bfloat16
    AX = mybir.AxisListType.X
    ALU = mybir.AluOpType
    ACTF = mybir.ActivationFunctionType

    assert rows % P == 0 and cols % wtile == 0
    nchunks = rows // P
    ntiles = cols // wtile
    ncols_f = float(cols)

    # Bacc (not raw Bass): its compile() runs generate_event_semaphores,
    # which splits multi-wait sync_info to satisfy the TRN2 1-wait/inst limit.
    nc = bacc.Bacc()
    x = nc.declare_dram_parameter("x", [rows, cols], f32, isOutput=False)
    out = nc.declare_dram_parameter("out", [rows, cols], f32, isOutput=True)

    with tile.TileContext(nc) as tc:
        with (
            tc.tile_pool(name="xp", bufs=xbufs) as xp,
            tc.tile_pool(name="tp", bufs=1) as tp,
            tc.tile_pool(name="sp", bufs=3) as sp,
        ):
            # trash output for the ACT stats pass; values never read
            # (only the f32 accumulator matters), so bf16 to save SBUF.
            trash = tp.tile([P, wtile], bf16, name="trash")
            for c in range(nchunks):
                r0 = c * P
                xt = [
                    xp.tile([P, wtile], f32, tag="xt", name=f"xt{c}_{j}")
                    for j in range(ntiles)
                ]
                mx = sp.tile([P, ntiles], f32, tag="mx", name=f"mx{c}")
                mn = sp.tile([P, ntiles], f32, tag="mn", name=f"mn{c}")
                sx = sp.tile([P, ntiles], f32, tag="sx", name=f"sx{c}")
                sq = sp.tile([P, ntiles], f32, tag="sq", name=f"sq{c}")
                xmax = sp.tile([P, 1], f32, tag="xmax", name=f"xmax{c}")
                xmin = sp.tile([P, 1], f32, tag="xmin", name=f"xmin{c}")
                bb = sp.tile([P, 1], f32, tag="bb", name=f"bb{c}")
                sxt = sp.tile([P, 1], f32, tag="sxt", name=f"sxt{c}")
                sqt = sp.tile([P, 1], f32, tag="sqt", name=f"sqt{c}")
                ssum = sp.tile([P, 1], f32, tag="ssum", name=f"ssum{c}")
                qq = sp.tile([P, 1], f32, tag="qq", name=f"qq{c}")
                sca = sp.tile([P, 1], f32, tag="sca", name=f"sca{c}")
                bia = sp.tile([P, 1], f32, tag="bia", name=f"bia{c}")

                for j in range(ntiles):
                    nc.sync.dma_start(
                        out=xt[j], in_=x[r0 : r0 + P, j * wtile : (j + 1) * wtile]
                    )
                for j in range(ntiles):
                    # row stats via in-place x+0 tensor_scalar (2x f32 mode);
                    # op1 selects the accumulate reduction (add/max/min).
                    for stat_tile, stat_op in ((sx, ALU.add), (mx, ALU.max), (mn, ALU.min)):
                        nc.vector.tensor_scalar(
                            out=xt[j],
                            in0=xt[j],
                            scalar1=0.0,
                            scalar2=None,
                            op0=ALU.add,
                            op1=stat_op,
                            accum_out=stat_tile[:, j : j + 1],
                        )
                    # 0.25*sum(x^2): Square(0.5 x) with accumulate, trash out
                    nc.scalar.activation(
                        out=trash,
                        in_=xt[j],
                        func=ACTF.Square,
                        bias=0.0,
                        scale=0.5,
                        accum_out=sq[:, j : j + 1],
                    )
                nc.vector.tensor_reduce(out=xmax, in_=mx, axis=AX, op=ALU.max)
                nc.vector.tensor_reduce(out=xmin, in_=mn, axis=AX, op=ALU.min)
                nc.vector.tensor_reduce(out=sxt, in_=sx, axis=AX, op=ALU.add)
                nc.vector.tensor_reduce(out=sqt, in_=sq, axis=AX, op=ALU.add)
                # b = 0.5*xmax + 1 - xmin
                nc.vector.tensor_scalar(
                    out=bb,
                    in0=xmax,
                    scalar1=0.5,
                    scalar2=1.0,
                    op0=ALU.mult,
                    op1=ALU.add,
                )
                nc.vector.tensor_tensor(out=bb, in0=bb, in1=xmin, op=ALU.subtract)
                # S = sqt + b*sxt + N*b^2 + 1e-12
                nc.vector.tensor_tensor(out=sxt, in0=bb, in1=sxt, op=ALU.mult)
                nc.vector.tensor_tensor(out=ssum, in0=bb, in1=bb, op=ALU.mult)
                nc.vector.tensor_scalar(
                    out=ssum, in0=ssum, scalar1=ncols_f, scalar2=None, op0=ALU.mult
                )
                nc.vector.tensor_tensor(out=ssum, in0=ssum, in1=sxt, op=ALU.add)
                nc.vector.tensor_tensor(out=ssum, in0=ssum, in1=sqt, op=ALU.add)
                nc.vector.tensor_scalar(
                    out=ssum, in0=ssum, scalar1=1e-12, scalar2=None, op0=ALU.add
                )
                # q = sqrt(1/S); scale' = 0.5q, bias' = q*b
                nc.vector.reciprocal(out=qq, in_=ssum)
                nc.scalar.activation(out=qq, in_=qq, func=ACTF.Sqrt)
                nc.vector.tensor_scalar(
                    out=sca, in0=qq, scalar1=0.5, scalar2=None, op0=ALU.mult
                )
                nc.vector.tensor_tensor(out=bia, in0=qq, in1=bb, op=ALU.mult)
                # out = (0.5q*x + q*b)^2 in place, then store
                for j in range(ntiles):
                    nc.scalar.activation(
                        out=xt[j],
                        in_=xt[j],
                        func=ACTF.Square,
                        bias=bia,
                        scale=sca,
                    )
                    nc.sync.dma_start(
                        out=out[r0 : r0 + P, j * wtile : (j + 1) * wtile], in_=xt[j]
                    )
    # Run Bacc passes (register allocation + the 1-wait/inst sync split).
    # run_bass_via_pjrt serializes nc as-is and never finalizes prebuilt
    # modules; without this walrus crashes on unallocated virtual registers.
    nc.finalize()
    return nc


def kernel(x: np.ndarray) -> np.ndarray:
    from concourse.bass_utils import run_bass_kernel_spmd

    x = np.ascontiguousarray(x, dtype=np.float32)
    assert x.shape == (ROWS, COLS)
    nc = _build(RPC, COLS, WTILE)
    in_maps = [{"x": x[i * RPC : (i + 1) * RPC]} for i in range(N_CORES)]
    res = run_bass_kernel_spmd(nc, in_maps, list(range(N_CORES)))
    return np.concatenate([r["out"] for r in res.results], axis=0)


# revision 8
# speedup vs baseline: 1.2951x; 1.2951x over previous
"""Entmax-1.5 (bisection reference) kernel for Trainium2, 8-core data parallel.

The reference runs 50 bisection iterations on tau with bracket
[min(xs)-1, max(xs)=0], xs = x - rowmax(x), z = 0.5*xs,
y = clip(z - tau, 0)^2, constraint = sum(y) - 1, and the update
  tmin = where(constraint < 0, tau, tmin)
  tmax = where(constraint > 0, tau, tmax)
For any row of width N >= 5 the first midpoint tau_1 = (min(xs)-1)/2
satisfies z_i - tau_1 = (xs_i - min(xs) + 1)/2 >= 1/2 for every i, so
constraint >= N/4 - 1 > 0 at tau_1 and at every later (smaller) tau.
Only tmax ever updates, and the f32 halving sequence collapses onto
tmin = min(xs) - 1 within ~30 iterations. Hence the reference equals

    w_i = (0.5*x_i + b)^2,  b = 0.5*rowmax(x) - rowmin(x) + 1
    out = w / (rowsum(w) + 1e-12)

(verified numerically: 5e-7 elementwise relative vs the 50-iter loop).

Kernel per core (512 rows x 32000 cols f32), per 128-row chunk of 8
column tiles (128 x 4000):
  DVE  rowmax + rowmin per tile (overlapped with the loads)
  ACT  w = Square(0.5x + b) in place with accumulated rowsum -> S
  DVE  r = 1/(S + 1e-12)
  out = w*r in place (tiles split between DVE tensor_scalar at 2x mode
  and ACT copy-with-scale, to shorten the serial tail), stores as each
  tile completes. 13 tile slots (5 spare) let the next chunk's loads
  run under this chunk's compute. One HBM read + one write, DMA-bound.
"""

import numpy as np

N_CORES = 8
ROWS, COLS = 4096, 32000
RPC = ROWS // N_CORES  # rows per core
P = 128  # SBUF partitions
WTILE = 4000  # column tile width
XBUFS = 13  # x-tile slots (each 128 x WTILE f32; SBUF is 224KB/partition)
DVE_SCALE_TILES = 4  # tiles of the final scale pass done on DVE (rest ACT)


def _build(rows, cols, wtile, xbufs=XBUFS):
    import concourse.bass as bass
    import concourse.tile as tile
    from concourse import bacc, mybir

    f32 = mybir.dt.float32
    AX = mybir.AxisListType.X
    ALU = mybir.AluOpType
    ACTF = mybir.ActivationFunctionType

    assert rows % P == 0 and cols % wtile == 0
    nchunks = rows // P
    ntiles = cols // wtile

    # Bacc (not raw Bass): its compile() runs generate_event_semaphores,
    # which splits multi-wait sync_info to satisfy the TRN2 1-wait/inst limit.
    nc = bacc.Bacc()
    x = nc.declare_dram_parameter("x", [rows, cols], f32, isOutput=False)
    out = nc.declare_dram_parameter("out", [rows, cols], f32, isOutput=True)

    with tile.TileContext(nc) as tc:
        with (
            tc.tile_pool(name="xp", bufs=xbufs) as xp,
            tc.tile_pool(name="sp", bufs=3) as sp,
        ):
            for c in range(nchunks):
                r0 = c * P
                xt = [
                    xp.tile([P, wtile], f32, tag="xt", name=f"xt{c}_{j}")
                    for j in range(ntiles)
                ]
                mx = sp.tile([P, ntiles], f32, tag="mx", name=f"mx{c}")
                mn = sp.tile([P, ntiles], f32, tag="mn", name=f"mn{c}")
                s = sp.tile([P, ntiles], f32, tag="s", name=f"s{c}")
                xmax = sp.tile([P, 1], f32, tag="xmax", name=f"xmax{c}")
                xmin = sp.tile([P, 1], f32, tag="xmin", name=f"xmin{c}")
                bias0 = sp.tile([P, 1], f32, tag="bias0", name=f"bias0{c}")
                ssum = sp.tile([P, 1], f32, tag="ssum", name=f"ssum{c}")
                rcp = sp.tile([P, 1], f32, tag="rcp", name=f"rcp{c}")

                for j in range(ntiles):
                    nc.sync.dma_start(
                        out=xt[j], in_=x[r0 : r0 + P, j * wtile : (j + 1) * wtile]
                    )
                for j in range(ntiles):
                    nc.vector.tensor_reduce(
                        out=mx[:, j : j + 1], in_=xt[j], axis=AX, op=ALU.max
                    )
                    nc.vector.tensor_reduce(
                        out=mn[:, j : j + 1], in_=xt[j], axis=AX, op=ALU.min
                    )
                nc.vector.tensor_reduce(out=xmax, in_=mx, axis=AX, op=ALU.max)
                nc.vector.tensor_reduce(out=xmin, in_=mn, axis=AX, op=ALU.min)
                # bias0 = 0.5*xmax + 1 - xmin
                nc.vector.tensor_scalar(
                    out=bias0,
                    in0=xmax,
                    scalar1=0.5,
                    scalar2=1.0,
                    op0=ALU.mult,
                    op1=ALU.add,
                )
                nc.vector.tensor_tensor(
                    out=bias0, in0=bias0, in1=xmin, op=ALU.subtract
                )
                # w = (0.5*x + bias0)^2 in place, with per-row sum
                for j in range(ntiles):
                    nc.scalar.activation(
                        out=xt[j],
                        in_=xt[j],
                        func=ACTF.Square,
                        bias=bias0,
                        scale=0.5,
                        accum_out=s[:, j : j + 1],
                    )
                nc.vector.tensor_reduce(out=ssum, in_=s, axis=AX, op=ALU.add)
                nc.vector.tensor_scalar(
                    out=ssum, in0=ssum, scalar1=1e-12, scalar2=None, op0=ALU.add
                )
                nc.vector.reciprocal(out=rcp, in_=ssum)
                # out = w * (1/S) in place, then store. Early tiles on DVE
                # (tensor_scalar runs 2x for f32), rest on ACT, so the two
                # engines drain the scale phase in parallel.
                for j in range(ntiles):
                    if j < DVE_SCALE_TILES:
                        nc.vector.tensor_scalar(
                            out=xt[j],
                            in0=xt[j],
                            scalar1=rcp,
                            scalar2=None,
                            op0=ALU.mult,
                        )
                    else:
                        nc.scalar.activation(
                            out=xt[j], in_=xt[j], func=ACTF.Copy, bias=0.0, scale=rcp
                        )
                    nc.sync.dma_start(
                        out=out[r0 : r0 + P, j * wtile : (j + 1) * wtile], in_=xt[j]
                    )
    # Run Bacc passes (register allocation + the 1-wait/inst sync split).
    # run_bass_via_pjrt serializes nc as-is and never finalizes prebuilt
    # modules; without this walrus crashes on unallocated virtual registers.
    nc.finalize()
    return nc


def kernel(x: np.ndarray) -> np.ndarray:
    from concourse.bass_utils import run_bass_kernel_spmd

    x = np.ascontiguousarray(x, dtype=np.float32)
    assert x.shape == (ROWS, COLS)
    nc = _build(RPC, COLS, WTILE)
    in_maps = [{"x": x[i * RPC : (i + 1) * RPC]} for i in range(N_CORES)]
    res = run_bass_kernel_spmd(nc, in_maps, list(range(N_CORES)))
    return np.concatenate([r["out"] for r in res.results], axis=0)
